# revision 1
# baseline (speedup 1.0000x reference)
# Trainium2 Bass kernel for nn_MeshUnpool (gnn_message_passing).
#
# Reference semantics (per mesh b):
#   idx = cumsum(dst_mask)-1 at true slots; padded[v,:] = mask[v] ? features[:,idx[v]] : 0
#   out = (unroll_mat[b].T @ padded).T / occ  ==  (features[b] @ unroll_mat[b][mask_rows]) / occ
# i.e. the gather+scatter collapses to selecting the E=3072 masked rows of
# unroll_mat, leaving a dense [NF,E] @ [E,U] matmul per mesh, divided
# column-wise by occurrences.  Pure data parallel: one mesh per core.
#
# On-device compute per core:
#   out[128, 4096] = sum_k (A_hi[k] + A_lo[k]).T @ W[k]  * inv_occ
# where A_hi/A_lo is a bf16 hi/lo split of features^T (f32-grade accuracy,
# since bf16*bf16 products are exact in the f32 PSUM accumulator) and W is the
# masked-row-gathered unroll matrix cast to bf16 (entries are exactly 0/1, so
# the cast is lossless and halves the dominant HBM traffic).

import numpy as np
import ml_dtypes

B, NF, E, U = 8, 128, 3072, 4096
NCORES = 8
KC = E // 128          # 24 contraction chunks of 128
NT = U // 512          # 8 output column tiles of 512 (one PSUM bank each)

_compiled = None


def _build_bass():
    import concourse.bass as bass
    import concourse.bacc as bacc
    import concourse.mybir as mybir
    import concourse.tile as tile

    nc = bacc.Bacc("TRN2", target_bir_lowering=False, debug=False)
    bf16 = mybir.dt.bfloat16
    f32 = mybir.dt.float32

    a_hi = nc.dram_tensor("a_hi", [128, E], bf16, kind="ExternalInput").ap()
    a_lo = nc.dram_tensor("a_lo", [128, E], bf16, kind="ExternalInput").ap()
    w = nc.dram_tensor("w", [E, U], bf16, kind="ExternalInput").ap()
    occ = nc.dram_tensor("occ", [128, U], f32, kind="ExternalInput").ap()
    out = nc.dram_tensor("out", [128, U], f32, kind="ExternalOutput").ap()

    with tile.TileContext(nc) as tc:
        with (
            tc.tile_pool(name="const", bufs=1) as cpool,
            tc.tile_pool(name="wpool", bufs=8) as wpool,
            tc.tile_pool(name="psum", bufs=2, space=bass.MemorySpace.PSUM) as ppool,
            tc.tile_pool(name="opool", bufs=3) as opool,
        ):
            # stationary operands + occ scale, loaded once
            a_hi_s = cpool.tile([128, E], bf16, tag="ahi")
            a_lo_s = cpool.tile([128, E], bf16, tag="alo")
            occ_s = cpool.tile([128, U], f32, tag="occ")
            nc.sync.dma_start(a_hi_s[:], a_hi)
            nc.sync.dma_start(a_lo_s[:], a_lo)
            nc.sync.dma_start(occ_s[:], occ)

            # host ships A^T chunk-interleaved: a_hi[p, k*128+m] = AT[k*128+p, m]
            # so chunk k's lhsT [K=128, M=128] is a_hi_s[:, k*128:(k+1)*128]
            for n in range(NT):
                psum = ppool.tile([128, 512], f32)
                for k in range(KC):
                    w_t = wpool.tile([128, 512], bf16, tag="w")
                    nc.sync.dma_start(
                        w_t[:], w[k * 128 : (k + 1) * 128, n * 512 : (n + 1) * 512]
                    )
                    nc.tensor.matmul(
                        psum[:],
                        a_hi_s[:, k * 128 : (k + 1) * 128],
                        w_t[:],
                        start=(k == 0),
                        stop=False,
                    )
                    nc.tensor.matmul(
                        psum[:],
                        a_lo_s[:, k * 128 : (k + 1) * 128],
                        w_t[:],
                        start=False,
                        stop=(k == KC - 1),
                    )
                o_t = opool.tile([128, 512], f32, tag="o")
                nc.vector.tensor_mul(
                    o_t[:], psum[:], occ_s[:, n * 512 : (n + 1) * 512]
                )
                nc.sync.dma_start(out[:, n * 512 : (n + 1) * 512], o_t[:])

    nc.compile()
    return nc


def _get_compiled():
    global _compiled
    if _compiled is None:
        _compiled = _build_bass()
    return _compiled


def _prep_core(features_b, unroll_b, occ_b, mask_b):
    bf16 = ml_dtypes.bfloat16
    # A^T = features^T [E, 128]; hi/lo bf16 split, chunk-interleaved to
    # [128, E] so SBUF partition p / free col k*128+m holds AT[k*128+p, m].
    at = np.ascontiguousarray(features_b.T.astype(np.float32))  # [E, 128]
    hi = at.astype(bf16)
    lo = (at - hi.astype(np.float32)).astype(bf16)

    def interleave(x):  # [E,128] -> [128,E] with the chunk layout above
        return np.ascontiguousarray(
            x.reshape(KC, 128, 128).transpose(1, 0, 2).reshape(128, KC * 128)
        )

    wg = unroll_b[mask_b].astype(bf16)  # [E, U]; 0/1 entries -> exact
    inv_occ = (1.0 / occ_b.reshape(U).astype(np.float32)).astype(np.float32)
    occ_bcast = np.ascontiguousarray(np.broadcast_to(inv_occ, (128, U)))
    return {
        "a_hi": interleave(hi),
        "a_lo": interleave(lo),
        "w": np.ascontiguousarray(wg),
        "occ": occ_bcast,
    }


def kernel(features, unroll_mat, occurrences, dst_masks):
    import concourse.bass_utils as bass_utils

    features = np.asarray(features, dtype=np.float32)
    unroll_mat = np.asarray(unroll_mat, dtype=np.float32)
    occurrences = np.asarray(occurrences, dtype=np.float32)
    dst_masks = np.asarray(dst_masks).astype(bool)

    nc = _get_compiled()
    in_maps = [
        _prep_core(features[b], unroll_mat[b], occurrences[b], dst_masks[b])
        for b in range(B)
    ]
    res = bass_utils.run_bass_kernel_spmd(nc, in_maps, core_ids=list(range(NCORES)))
    return np.stack([res.results[b]["out"] for b in range(B)], axis=0)


# revision 2
# speedup vs baseline: 1.4908x; 1.4908x over previous
# Trainium2 Bass kernel for nn_MeshUnpool (gnn_message_passing).
#
# Reference semantics (per mesh b):
#   idx = cumsum(dst_mask)-1 at true slots; padded[v,:] = mask[v] ? features[:,idx[v]] : 0
#   out = (unroll_mat[b].T @ padded).T / occ  ==  (features[b] @ unroll_mat[b][mask_rows]) / occ
# i.e. the gather+scatter collapses to selecting the E=3072 masked rows of
# unroll_mat, leaving a dense [NF,E] @ [E,U] matmul per mesh, divided
# column-wise by occurrences.  Pure data parallel: one mesh per core.
#
# On-device compute per core:
#   out[128, 4096] = sum_k (A_hi[k] + A_lo[k]).T @ W[k]  * inv_occ
# where A_hi/A_lo is a bf16 hi/lo split of features^T (f32-grade accuracy,
# since bf16*bf16 products are exact in the f32 PSUM accumulator) and W is the
# masked-row-gathered unroll matrix cast to bf16 (entries are exactly 0/1, so
# the cast is lossless and halves the dominant HBM traffic).

import numpy as np
import ml_dtypes

B, NF, E, U = 8, 128, 3072, 4096
NCORES = 8
KC = E // 128          # 24 contraction chunks of 128
NT = U // 512          # 8 output column tiles of 512 (one PSUM bank each)

_compiled = None


def _build_bass():
    import concourse.bass as bass
    import concourse.bacc as bacc
    import concourse.mybir as mybir
    import concourse.tile as tile

    nc = bacc.Bacc("TRN2", target_bir_lowering=False, debug=False)
    bf16 = mybir.dt.bfloat16
    f32 = mybir.dt.float32

    a_hi = nc.dram_tensor("a_hi", [128, E], bf16, kind="ExternalInput").ap()
    a_lo = nc.dram_tensor("a_lo", [128, E], bf16, kind="ExternalInput").ap()
    w = nc.dram_tensor("w", [E, U], bf16, kind="ExternalInput").ap()
    occ = nc.dram_tensor("occ", [128, U], f32, kind="ExternalInput").ap()
    out = nc.dram_tensor("out", [128, U], f32, kind="ExternalOutput").ap()

    with tile.TileContext(nc) as tc:
        with (
            tc.tile_pool(name="const", bufs=1) as cpool,
            tc.tile_pool(name="wpool", bufs=5) as wpool,
            tc.tile_pool(name="psum", bufs=1, space=bass.MemorySpace.PSUM) as ppool,
            tc.tile_pool(name="opool", bufs=3) as opool,
        ):
            # stationary operands, loaded once
            a_hi_s = cpool.tile([128, E], bf16, tag="ahi")
            a_lo_s = cpool.tile([128, E], bf16, tag="alo")
            occ_s = cpool.tile([128, U], f32, tag="occ")
            nc.sync.dma_start(a_hi_s[:], a_hi)
            nc.sync.dma_start(a_lo_s[:], a_lo)

            # all 8 PSUM banks accumulate in parallel; k-contiguous keeps PE warm
            psum = ppool.tile([128, U], f32, tag="ps")
            # host ships A^T chunk-interleaved: a_hi[p, k*128+m] = AT[k*128+p, m]
            # so chunk k's lhsT [K=128, M=128] is a_hi_s[:, k*128:(k+1)*128]
            for k in range(KC):
                w_t = wpool.tile([128, U], bf16, tag="w")
                nc.sync.dma_start(w_t[:], w[k * 128 : (k + 1) * 128, :])
                for half in range(2):  # 0: hi, 1: lo — same W tile, one load
                    a_s = a_hi_s if half == 0 else a_lo_s
                    for n in range(NT):
                        nc.tensor.matmul(
                            psum[:, n * 512 : (n + 1) * 512],
                            a_s[:, k * 128 : (k + 1) * 128],
                            w_t[:, n * 512 : (n + 1) * 512],
                            start=(k == 0 and half == 0),
                            stop=(k == KC - 1 and half == 1),
                        )
            # occ arrives late on purpose: it's only needed for the epilogue,
            # so its 2MB DMA must not delay the first W tiles
            nc.sync.dma_start(occ_s[:], occ)
            for n in range(NT):
                o_t = opool.tile([128, 512], f32, tag="o")
                nc.vector.tensor_mul(
                    o_t[:], psum[:, n * 512 : (n + 1) * 512],
                    occ_s[:, n * 512 : (n + 1) * 512],
                )
                nc.sync.dma_start(out[:, n * 512 : (n + 1) * 512], o_t[:])

    nc.compile()
    return nc


def _get_compiled():
    global _compiled
    if _compiled is None:
        _compiled = _build_bass()
    return _compiled


def _prep_core(features_b, unroll_b, occ_b, mask_b):
    bf16 = ml_dtypes.bfloat16
    # A^T = features^T [E, 128]; hi/lo bf16 split, chunk-interleaved to
    # [128, E] so SBUF partition p / free col k*128+m holds AT[k*128+p, m].
    at = np.ascontiguousarray(features_b.T.astype(np.float32))  # [E, 128]
    hi = at.astype(bf16)
    lo = (at - hi.astype(np.float32)).astype(bf16)

    def interleave(x):  # [E,128] -> [128,E] with the chunk layout above
        return np.ascontiguousarray(
            x.reshape(KC, 128, 128).transpose(1, 0, 2).reshape(128, KC * 128)
        )

    wg = unroll_b[mask_b].astype(bf16)  # [E, U]; 0/1 entries -> exact
    inv_occ = (1.0 / occ_b.reshape(U).astype(np.float32)).astype(np.float32)
    occ_bcast = np.ascontiguousarray(np.broadcast_to(inv_occ, (128, U)))
    return {
        "a_hi": interleave(hi),
        "a_lo": interleave(lo),
        "w": np.ascontiguousarray(wg),
        "occ": occ_bcast,
    }


def kernel(features, unroll_mat, occurrences, dst_masks):
    import concourse.bass_utils as bass_utils

    features = np.asarray(features, dtype=np.float32)
    unroll_mat = np.asarray(unroll_mat, dtype=np.float32)
    occurrences = np.asarray(occurrences, dtype=np.float32)
    dst_masks = np.asarray(dst_masks).astype(bool)

    nc = _get_compiled()
    in_maps = [
        _prep_core(features[b], unroll_mat[b], occurrences[b], dst_masks[b])
        for b in range(B)
    ]
    res = bass_utils.run_bass_kernel_spmd(nc, in_maps, core_ids=list(range(NCORES)))
    return np.stack([res.results[b]["out"] for b in range(B)], axis=0)


# revision 6
# speedup vs baseline: 1.5022x; 1.0077x over previous
# Trainium2 Bass kernel for nn_MeshUnpool (gnn_message_passing).
#
# Reference semantics (per mesh b):
#   idx = cumsum(dst_mask)-1 at true slots; padded[v,:] = mask[v] ? features[:,idx[v]] : 0
#   out = (unroll_mat[b].T @ padded).T / occ  ==  (features[b] @ unroll_mat[b][mask_rows]) / occ
# i.e. the gather+scatter collapses to selecting the E=3072 masked rows of
# unroll_mat, leaving a dense [NF,E] @ [E,U] matmul per mesh, divided
# column-wise by occurrences.  Pure data parallel: one mesh per core.
#
# On-device compute per core:
#   out[128, 4096] = sum_k (A_hi[k] + A_lo[k]).T @ W[k]  * inv_occ
# where A_hi/A_lo is a bf16 hi/lo split of features^T (f32-grade accuracy,
# since bf16*bf16 products are exact in the f32 PSUM accumulator) and W is the
# masked-row-gathered unroll matrix cast to bf16 (entries are exactly 0/1, so
# the cast is lossless and halves the dominant HBM traffic).

import numpy as np
import ml_dtypes

B, NF, E, U = 8, 128, 3072, 4096
NCORES = 8
KC = E // 128          # 24 contraction chunks of 128
NT = U // 512          # 8 output column tiles of 512 (one PSUM bank each)

_compiled = None


def _build_bass():
    import concourse.bass as bass
    import concourse.bacc as bacc
    import concourse.mybir as mybir
    import concourse.tile as tile

    nc = bacc.Bacc("TRN2", target_bir_lowering=False, debug=False)
    bf16 = mybir.dt.bfloat16
    f32 = mybir.dt.float32

    a_hi = nc.dram_tensor("a_hi", [128, E], bf16, kind="ExternalInput").ap()
    a_lo = nc.dram_tensor("a_lo", [128, E], bf16, kind="ExternalInput").ap()
    w = nc.dram_tensor("w", [E, U], bf16, kind="ExternalInput").ap()
    occ = nc.dram_tensor("occ", [128, U], f32, kind="ExternalInput").ap()
    out = nc.dram_tensor("out", [128, U], f32, kind="ExternalOutput").ap()

    with tile.TileContext(nc) as tc:
        with (
            tc.tile_pool(name="const", bufs=1) as cpool,
            tc.tile_pool(name="wpool", bufs=5) as wpool,
            tc.tile_pool(name="psum", bufs=1, space=bass.MemorySpace.PSUM) as ppool,
            tc.tile_pool(name="opool", bufs=3) as opool,
        ):
            # stationary operands on the scalar HWDGE ring so the sync ring
            # streams W exclusively
            a_hi_s = cpool.tile([128, E], bf16, tag="ahi")
            a_lo_s = cpool.tile([128, E], bf16, tag="alo")
            occ_s = cpool.tile([128, U], f32, tag="occ")
            nc.scalar.dma_start(a_hi_s[:], a_hi)
            nc.scalar.dma_start(a_lo_s[:], a_lo)

            # all 8 PSUM banks accumulate in parallel; k-contiguous keeps PE warm
            psum = ppool.tile([128, U], f32, tag="ps")
            # host ships A^T chunk-interleaved: a_hi[p, k*128+m] = AT[k*128+p, m]
            # so chunk k's lhsT [K=128, M=128] is a_hi_s[:, k*128:(k+1)*128]
            for k in range(KC):
                w_t = wpool.tile([128, U], bf16, tag="w")
                nc.sync.dma_start(w_t[:], w[k * 128 : (k + 1) * 128, :])
                if k == KC // 2:
                    # occ is only needed for the epilogue; mid-stream the DMA
                    # slack behind the PE-bound phase absorbs it for free
                    nc.scalar.dma_start(occ_s[:], occ)
                for half in range(2):  # 0: hi, 1: lo — same W tile, one load
                    a_s = a_hi_s if half == 0 else a_lo_s
                    for n in range(NT):
                        nc.tensor.matmul(
                            psum[:, n * 512 : (n + 1) * 512],
                            a_s[:, k * 128 : (k + 1) * 128],
                            w_t[:, n * 512 : (n + 1) * 512],
                            start=(k == 0 and half == 0),
                            stop=(k == KC - 1 and half == 1),
                        )
            for n in range(NT):
                o_t = opool.tile([128, 512], f32, tag="o")
                nc.vector.tensor_mul(
                    o_t[:], psum[:, n * 512 : (n + 1) * 512],
                    occ_s[:, n * 512 : (n + 1) * 512],
                )
                nc.sync.dma_start(out[:, n * 512 : (n + 1) * 512], o_t[:])

    nc.compile()
    _dedup_ldweights(nc)
    return nc


def _dedup_ldweights(nc):
    """Remove InstLdweights that reload the PE array with the exact weights it
    already holds (consecutive matmuls sharing one stationary operand).  The
    tile legalizer emits one LDWEIGHTS per matmul and neither it nor walrus
    dedups, so 8-matmul groups sharing a lhsT pay 7 redundant ~100ns array
    loads each — pure serial PE time.  Safe here because the stationary tiles
    (bufs=1, written once) are never rewritten mid-kernel.  Any waits/updates
    on a removed LDW are transferred to the next PE instruction."""
    import concourse.mybir as mybir

    for blk in nc.m.functions[0].blocks:
        insts = blk.instructions
        loaded = None
        pending = []  # sync infos of removed LDWs, to merge into next PE inst
        idx = 0
        removed = 0
        while idx < len(insts):
            inst = insts[idx]
            if isinstance(inst, mybir.InstLdweights):
                key = (
                    str(inst.ins[0]),
                    str(inst.tile_position),
                    str(inst.perf_mode),
                    str(inst.is_transpose),
                )
                if loaded == key:
                    si = inst.sync_info
                    if si is not None and (si.on_wait or si.on_update):
                        pending.append(si)
                    del insts[idx]
                    removed += 1
                    continue
                loaded = key
            elif isinstance(inst, mybir.InstMatmult) and pending:
                si = inst.sync_info
                if si is None:
                    si = mybir.SyncInfo(on_wait=[], on_update=[])
                for p in pending:
                    si.on_wait = list(si.on_wait) + list(p.on_wait)
                    si.on_update = list(si.on_update) + list(p.on_update)
                inst.sync_info = si
                pending = []
            idx += 1
        assert not pending, "dangling sync from removed LDWEIGHTS"


def _get_compiled():
    global _compiled
    if _compiled is None:
        _compiled = _build_bass()
    return _compiled


def _prep_core(features_b, unroll_b, occ_b, mask_b):
    bf16 = ml_dtypes.bfloat16
    # A^T = features^T [E, 128]; hi/lo bf16 split, chunk-interleaved to
    # [128, E] so SBUF partition p / free col k*128+m holds AT[k*128+p, m].
    at = np.ascontiguousarray(features_b.T.astype(np.float32))  # [E, 128]
    hi = at.astype(bf16)
    lo = (at - hi.astype(np.float32)).astype(bf16)

    def interleave(x):  # [E,128] -> [128,E] with the chunk layout above
        return np.ascontiguousarray(
            x.reshape(KC, 128, 128).transpose(1, 0, 2).reshape(128, KC * 128)
        )

    wg = unroll_b[mask_b].astype(bf16)  # [E, U]; 0/1 entries -> exact
    inv_occ = (1.0 / occ_b.reshape(U).astype(np.float32)).astype(np.float32)
    occ_bcast = np.ascontiguousarray(np.broadcast_to(inv_occ, (128, U)))
    return {
        "a_hi": interleave(hi),
        "a_lo": interleave(lo),
        "w": np.ascontiguousarray(wg),
        "occ": occ_bcast,
    }


def kernel(features, unroll_mat, occurrences, dst_masks):
    import concourse.bass_utils as bass_utils

    features = np.asarray(features, dtype=np.float32)
    unroll_mat = np.asarray(unroll_mat, dtype=np.float32)
    occurrences = np.asarray(occurrences, dtype=np.float32)
    dst_masks = np.asarray(dst_masks).astype(bool)

    nc = _get_compiled()
    in_maps = [
        _prep_core(features[b], unroll_mat[b], occurrences[b], dst_masks[b])
        for b in range(B)
    ]
    res = bass_utils.run_bass_kernel_spmd(nc, in_maps, core_ids=list(range(NCORES)))
    return np.stack([res.results[b]["out"] for b in range(B)], axis=0)
